# revision 19
# baseline (speedup 1.0000x reference)
"""Trainium2 Bass kernel for nn_Attention2d (sparse_attention).

Math (per reference):
  x: (2, 128, 64, 64); T = 4096 tokens; 4 heads x 32 channels.
  qkv 1x1-conv -> per-head attention over T -> 1x1-conv out proj -> residual.

Sharding: one (batch, head) pair per core (8 cores). Each core computes its
head's attention fully on-chip (flash-style streaming; no max-subtraction --
scores are O(7) so plain exp is safe) and returns the per-head partial of
the output projection; the host sums the 4 head partials per batch and adds
the residual + biases (exact).

Per-core structure (T=4096, t-blocks of 512, s-supers of 4x128):
  - q,k replicated x4 across partition groups so the K=32 score matmuls
    row-pack 4-wide into the PE array (tile_position).
  - Score blocks are written to PSUM as bf16: a full super (4 x 128s x 512t)
    fits in 2 banks, so score tiles triple-buffer in 6 banks and ScalarE's
    exp (the pacer, 1 elem/cycle/lane, 16.7M exps/core) never blocks the
    next super's score matmuls.
  - v is produced directly transposed (x_block stationary) with a 32-wide
    ones block appended: each PV matmul also emits the softmax denominator
    on partitions 32..63.
  - PV accumulates a whole t-block (32 matmuls) into a 1-bank fp32 PSUM
    accumulator (2 rotating banks): no VectorE work on the critical path.
  - Projections / vT / output-projection ride inside the super loop via the
    spare score-tag slots, so prologue and tail stay short.
"""

import numpy as np
import ml_dtypes

B, C, Hh, Ww = 2, 128, 64, 64
T = Hh * Ww          # 4096
NH, CH = 4, 32
SCALE2 = float(1.0 / np.sqrt(CH))
N_CORES = 8
NSUP = T // 512      # 8 supers per t-block, 8 t-blocks

_cache = {}


def _build_nc(debug=False):
    import concourse.tile as tile
    from concourse import bacc, mybir

    BF16 = mybir.dt.bfloat16
    F32 = mybir.dt.float32
    Exp = mybir.ActivationFunctionType.Exp

    nc = bacc.Bacc("TRN2", target_bir_lowering=False, debug=False,
                   num_devices=N_CORES)
    dbg = {}
    if debug:
        dbg["q"] = nc.dram_tensor("dq", [128, T], BF16, kind="ExternalOutput")
        dbg["k"] = nc.dram_tensor("dk", [128, T], BF16, kind="ExternalOutput")
        dbg["vT"] = nc.dram_tensor("dvT", [128, 2048], BF16, kind="ExternalOutput")
        dbg["an"] = nc.dram_tensor("dan", [32, T], BF16, kind="ExternalOutput")
        dbg["acc"] = nc.dram_tensor("dacc", [64, 512], F32, kind="ExternalOutput")
        dbg["p"] = nc.dram_tensor("dp", [128, 2048], BF16, kind="ExternalOutput")

    x_in = nc.dram_tensor("x", [128, T], BF16, kind="ExternalInput")
    wq_in = nc.dram_tensor("wqT", [128, 128], BF16, kind="ExternalInput")
    wk_in = nc.dram_tensor("wkT", [128, 128], BF16, kind="ExternalInput")
    wv_in = nc.dram_tensor("wvT", [128, 32], BF16, kind="ExternalInput")
    wp_in = nc.dram_tensor("wpT", [32, 128], BF16, kind="ExternalInput")
    bq_in = nc.dram_tensor("bq", [128, 1], F32, kind="ExternalInput")
    bk_in = nc.dram_tensor("bk", [128, 1], F32, kind="ExternalInput")
    out_t = nc.dram_tensor("out", [128, T], F32, kind="ExternalOutput")

    with tile.TileContext(nc) as tc:
        with (
            tc.tile_pool(name="const", bufs=1) as cpool,
            tc.tile_pool(name="work", bufs=2) as wpool,
            tc.tile_pool(name="psum", bufs=1, space="PSUM") as pspool,
        ):
            x_sb = cpool.tile([128, T], BF16)
            for c in range(4):
                nc.sync.dma_start(x_sb[:, c * 1024:(c + 1) * 1024],
                                  x_in[:, c * 1024:(c + 1) * 1024])
            wq_sb = cpool.tile([128, 128], BF16)
            nc.sync.dma_start(wq_sb[:], wq_in[:])
            wk_sb = cpool.tile([128, 128], BF16)
            nc.sync.dma_start(wk_sb[:], wk_in[:])
            wv_sb = cpool.tile([128, 32], BF16)
            nc.sync.dma_start(wv_sb[:], wv_in[:])
            wp_sb = cpool.tile([32, 128], BF16)
            nc.sync.dma_start(wp_sb[:], wp_in[:])
            bq_sb = cpool.tile([128, 1], F32)
            nc.sync.dma_start(bq_sb[:], bq_in[:])
            bk_sb = cpool.tile([128, 1], F32)
            nc.sync.dma_start(bk_sb[:], bk_in[:])

            q_sb = cpool.tile([128, T], BF16)
            k_sb = cpool.tile([128, T], BF16)
            vT_sb = cpool.tile([128, 64 * (T // 128)], BF16)  # (128, 2048)
            an_sb = cpool.tile([32, T], BF16)

            nc.gpsimd.memset(vT_sb[:], 1.0)

            # scratch fp32 1-bank tiles share the score tag's slots
            def scratch(nm):
                return pspool.tile([128, 512], F32, tag="st", bufs=1, name=nm)

            def emit_proj(wsb, bsb, dst, c, nm):
                ps = scratch(nm)
                nc.tensor.matmul(ps[:], wsb[:],
                                 x_sb[:, c * 512:(c + 1) * 512],
                                 start=True, stop=True)
                nc.vector.tensor_scalar_add(
                    dst[:, c * 512:(c + 1) * 512], ps[:], bsb[:])

            def emit_vt(half):
                ps = scratch(f"pp_v{half}")
                for j16 in range(16):
                    j = half * 16 + j16
                    nc.tensor.matmul(
                        ps[:, j16 * 32:(j16 + 1) * 32],
                        x_sb[:, j * 128:(j + 1) * 128],
                        wv_sb[:],
                        start=True, stop=True)
                src = ps[:].rearrange("p (j c) -> p j c", c=32)
                dstv = vT_sb[:].rearrange("p (j c) -> p j c", c=64)
                nc.vector.tensor_copy(
                    dstv[:, half * 16:(half + 1) * 16, 0:32], src)

            # ---- prologue: only what super 0/1 needs ----
            for c in range(4):
                emit_proj(wk_sb, bk_sb, k_sb, c, f"pp_k{c}")
            emit_proj(wq_sb, bq_sb, q_sb, 0, "pp_q0")
            emit_vt(0)

            if debug:
                nc.sync.dma_start(dbg["q"][:], q_sb[:])
                nc.sync.dma_start(dbg["k"][:], k_sb[:])
                nc.sync.dma_start(dbg["vT"][:], vT_sb[:])

            # ---- attention, software-pipelined over 64 supers ----
            state = {}
            pv_tiles = {}

            def emit_score_exp(jg):
                tb, j = divmod(jg, NSUP)
                if j == 0:
                    pv_tiles[tb] = pspool.tile(
                        [64, 512], F32, tag="pv", bufs=2, name=f"pv_{tb}")
                st = pspool.tile([128, 2048], F32, tag="st", bufs=1,
                                 name=f"st_{jg}")
                tsl = slice(tb * 512, (tb + 1) * 512)
                for g in range(4):
                    sblk = 4 * j + g
                    nc.tensor.matmul(
                        st[:, g * 512:(g + 1) * 512],
                        k_sb[32 * g:32 * (g + 1), 128 * sblk:128 * (sblk + 1)],
                        q_sb[32 * g:32 * (g + 1), tsl],
                        start=True, stop=True,
                        tile_position=(32 * g, 0))
                p_sb = wpool.tile([128, 2048], BF16, tag="p")
                nc.scalar.activation(p_sb[:], st[:], Exp, scale=SCALE2)
                if debug and jg == 0:
                    nc.sync.dma_start(dbg["p"][:], p_sb[:])
                state[jg] = p_sb

            def emit_pv(jg):
                tb, j = divmod(jg, NSUP)
                p_sb = state.pop(jg)
                pv = pv_tiles[tb]
                for g in range(4):
                    sblk = 4 * j + g
                    nc.tensor.matmul(
                        pv[:], vT_sb[:, 64 * sblk:64 * (sblk + 1)],
                        p_sb[:, g * 512:(g + 1) * 512],
                        start=(j == 0 and g == 0), stop=(j == NSUP - 1 and g == 3),
                        skip_group_check=True)
                if j == NSUP - 1:
                    # t-block epilogue + its output-projection chunk (VectorE
                    # + one PE matmul; all off the critical path)
                    a_sb = wpool.tile([64, 512], F32, tag="acc")
                    nc.vector.tensor_copy(a_sb[:], pv[:])
                    if debug and tb == 0:
                        nc.sync.dma_start(dbg["acc"][:], a_sb[:])
                    tsl = slice(tb * 512, (tb + 1) * 512)
                    dcp = wpool.tile([32, 512], F32, tag="dcp")
                    nc.vector.tensor_copy(dcp[:], a_sb[32:64, :])
                    rc = wpool.tile([32, 512], F32, tag="rc")
                    sc = wpool.tile([32, 512], F32, tag="sc")
                    nc.vector.reciprocal_approx_accurate(rc[:], dcp[:], sc[:])
                    nc.vector.tensor_mul(an_sb[:, tsl], a_sb[0:32, :], rc[:])
                    op = scratch(f"pp_o{tb}")
                    nc.tensor.matmul(op[:], wp_sb[:], an_sb[:, tsl],
                                     start=True, stop=True)
                    o_sb = wpool.tile([128, 512], F32, tag="o")
                    nc.vector.tensor_copy(o_sb[:], op[:])
                    nc.sync.dma_start(out_t[:, tsl], o_sb[:])

            for jg in range(NSUP * NSUP):
                emit_score_exp(jg)
                # stagger remaining input prep into the early supers
                if jg == 0:
                    emit_vt(1)
                elif 1 <= jg <= 3:
                    emit_proj(wk_sb, bk_sb, k_sb, jg + 3, f"pp_k{jg + 3}")
                elif jg == 4:
                    emit_proj(wk_sb, bk_sb, k_sb, 7, "pp_k7")
                    emit_proj(wq_sb, bq_sb, q_sb, 1, "pp_q1")
                elif 5 <= jg <= 10:
                    emit_proj(wq_sb, bq_sb, q_sb, jg - 3, f"pp_q{jg - 3}")
                if jg >= 1:
                    emit_pv(jg - 1)
            emit_pv(NSUP * NSUP - 1)
            if debug:
                nc.sync.dma_start(dbg["an"][:], an_sb[:])

    nc.compile()
    return nc


def _get_nc(debug=False):
    key = ("nc", debug)
    if key not in _cache:
        _cache[key] = _build_nc(debug)
    return _cache[key]


def _make_in_maps(x_, w_qkv, b_qkv, w_proj):
    bf16 = ml_dtypes.bfloat16
    in_maps = []
    for core in range(N_CORES):
        b, g = divmod(core, NH)
        wq = w_qkv[96 * g:96 * g + 32]
        wk = w_qkv[96 * g + 32:96 * g + 64]
        wv = w_qkv[96 * g + 64:96 * g + 96]
        in_maps.append({
            "x": x_[b].astype(bf16),
            "wqT": np.ascontiguousarray(np.tile(wq, (4, 1)).T).astype(bf16),
            "wkT": np.ascontiguousarray(np.tile(wk, (4, 1)).T).astype(bf16),
            "wvT": np.ascontiguousarray(wv.T).astype(bf16),
            "wpT": np.ascontiguousarray(
                w_proj[:, 32 * g:32 * (g + 1)].T).astype(bf16),
            "bq": np.ascontiguousarray(
                np.tile(b_qkv[96 * g:96 * g + 32], 4).reshape(128, 1)),
            "bk": np.ascontiguousarray(
                np.tile(b_qkv[96 * g + 32:96 * g + 64], 4).reshape(128, 1)),
        })
    return in_maps


def _run(x, w_qkv, b_qkv, w_proj, b_proj, trace=False):
    from concourse.bass_utils import run_bass_kernel_spmd

    nc = _get_nc()
    x_ = np.ascontiguousarray(np.asarray(x, np.float32).reshape(B, C, T))
    w_qkv = np.asarray(w_qkv, np.float32)
    b_qkv = np.asarray(b_qkv, np.float32)
    w_proj = np.asarray(w_proj, np.float32)
    b_proj = np.asarray(b_proj, np.float32)

    in_maps = _make_in_maps(x_, w_qkv, b_qkv, w_proj)
    res = run_bass_kernel_spmd(nc, in_maps, core_ids=list(range(N_CORES)),
                               trace=trace)
    out = np.empty((B, C, T), np.float32)
    for b in range(B):
        acc = x_[b] + b_proj[:, None]
        for g in range(NH):
            wp = w_proj[:, 32 * g:32 * (g + 1)]
            bv = b_qkv[96 * g + 64:96 * g + 96]
            acc = acc + res.results[NH * b + g]["out"] + (wp @ bv)[:, None]
        out[b] = acc
    return out.reshape(B, C, Hh, Ww), res


def kernel(x, w_qkv, b_qkv, w_proj, b_proj):
    out, _ = _run(x, w_qkv, b_qkv, w_proj, b_proj, trace=False)
    return out.astype(np.asarray(x).dtype)


# revision 20
# speedup vs baseline: 1.2552x; 1.2552x over previous
"""Trainium2 Bass kernel for nn_Attention2d (sparse_attention).

Math (per reference):
  x: (2, 128, 64, 64); T = 4096 tokens; 4 heads x 32 channels.
  qkv 1x1-conv -> per-head attention over T -> 1x1-conv out proj -> residual.

Sharding: one (batch, head) pair per core (8 cores). Each core computes its
head's attention fully on-chip (flash-style streaming; no max-subtraction --
scores are O(7) so plain exp is safe) and returns the per-head partial of
the output projection; the host sums the 4 head partials per batch and adds
the residual + biases (exact).

Per-core structure (T=4096, t-blocks of 512, s-supers of 4x128):
  - q,k replicated x4 across partition groups so the K=32 score matmuls
    row-pack 4-wide into the PE array (tile_position).
  - Score blocks are written to PSUM as bf16: a full super (4 x 128s x 512t)
    fits in 2 banks, so score tiles triple-buffer in 6 banks and ScalarE's
    exp (the pacer, 1 elem/cycle/lane, 16.7M exps/core) never blocks the
    next super's score matmuls.
  - v is produced directly transposed (x_block stationary) with a 32-wide
    ones block appended: each PV matmul also emits the softmax denominator
    on partitions 32..63.
  - PV accumulates a whole t-block (32 matmuls) into a 1-bank fp32 PSUM
    accumulator (2 rotating banks): no VectorE work on the critical path.
  - Projections / vT / output-projection ride inside the super loop via the
    spare score-tag slots, so prologue and tail stay short.
"""

import numpy as np
import ml_dtypes

B, C, Hh, Ww = 2, 128, 64, 64
T = Hh * Ww          # 4096
NH, CH = 4, 32
SCALE2 = float(1.0 / np.sqrt(CH))
N_CORES = 8
NSUP = T // 512      # 8 supers per t-block, 8 t-blocks

_cache = {}


def _build_nc(debug=False):
    import concourse.tile as tile
    from concourse import bacc, mybir

    BF16 = mybir.dt.bfloat16
    F32 = mybir.dt.float32
    Exp = mybir.ActivationFunctionType.Exp

    nc = bacc.Bacc("TRN2", target_bir_lowering=False, debug=False,
                   num_devices=N_CORES)
    dbg = {}
    if debug:
        dbg["q"] = nc.dram_tensor("dq", [128, T], BF16, kind="ExternalOutput")
        dbg["k"] = nc.dram_tensor("dk", [128, T], BF16, kind="ExternalOutput")
        dbg["vT"] = nc.dram_tensor("dvT", [128, 2048], BF16, kind="ExternalOutput")
        dbg["an"] = nc.dram_tensor("dan", [32, T], BF16, kind="ExternalOutput")
        dbg["acc"] = nc.dram_tensor("dacc", [64, 512], F32, kind="ExternalOutput")
        dbg["p"] = nc.dram_tensor("dp", [128, 2048], BF16, kind="ExternalOutput")

    x_in = nc.dram_tensor("x", [128, T], BF16, kind="ExternalInput")
    wq_in = nc.dram_tensor("wqT", [128, 128], BF16, kind="ExternalInput")
    wk_in = nc.dram_tensor("wkT", [128, 128], BF16, kind="ExternalInput")
    wv_in = nc.dram_tensor("wvT", [128, 32], BF16, kind="ExternalInput")
    wp_in = nc.dram_tensor("wpT", [32, 128], BF16, kind="ExternalInput")
    bq_in = nc.dram_tensor("bq", [128, 1], F32, kind="ExternalInput")
    bk_in = nc.dram_tensor("bk", [128, 1], F32, kind="ExternalInput")
    out_t = nc.dram_tensor("out", [128, T], F32, kind="ExternalOutput")

    with tile.TileContext(nc) as tc:
        with (
            tc.tile_pool(name="const", bufs=1) as cpool,
            tc.tile_pool(name="work", bufs=2) as wpool,
            tc.tile_pool(name="psum", bufs=1, space="PSUM") as pspool,
        ):
            x_sb = cpool.tile([128, T], BF16)
            for c in range(4):
                nc.sync.dma_start(x_sb[:, c * 1024:(c + 1) * 1024],
                                  x_in[:, c * 1024:(c + 1) * 1024])
            wq_sb = cpool.tile([128, 128], BF16)
            nc.sync.dma_start(wq_sb[:], wq_in[:])
            wk_sb = cpool.tile([128, 128], BF16)
            nc.sync.dma_start(wk_sb[:], wk_in[:])
            wv_sb = cpool.tile([128, 32], BF16)
            nc.sync.dma_start(wv_sb[:], wv_in[:])
            wp_sb = cpool.tile([32, 128], BF16)
            nc.sync.dma_start(wp_sb[:], wp_in[:])
            bq_sb = cpool.tile([128, 1], F32)
            nc.sync.dma_start(bq_sb[:], bq_in[:])
            bk_sb = cpool.tile([128, 1], F32)
            nc.sync.dma_start(bk_sb[:], bk_in[:])

            q_sb = cpool.tile([128, T], BF16)
            k_sb = cpool.tile([128, T], BF16)
            vT_sb = cpool.tile([128, 64 * (T // 128)], BF16)  # (128, 2048)
            an_sb = cpool.tile([32, T], BF16)

            nc.gpsimd.memset(vT_sb[:], 1.0)

            # 1-bank fp32 ping-pong tiles for projections / vT / out-proj
            def scratch(nm):
                return pspool.tile([128, 512], F32, tag="pp", bufs=2, name=nm)

            def emit_proj(wsb, bsb, dst, c, nm):
                ps = scratch(nm)
                nc.tensor.matmul(ps[:], wsb[:],
                                 x_sb[:, c * 512:(c + 1) * 512],
                                 start=True, stop=True)
                nc.vector.tensor_scalar_add(
                    dst[:, c * 512:(c + 1) * 512], ps[:], bsb[:])

            def emit_vt(half):
                ps = scratch(f"pp_v{half}")
                for j16 in range(16):
                    j = half * 16 + j16
                    nc.tensor.matmul(
                        ps[:, j16 * 32:(j16 + 1) * 32],
                        x_sb[:, j * 128:(j + 1) * 128],
                        wv_sb[:],
                        start=True, stop=True)
                src = ps[:].rearrange("p (j c) -> p j c", c=32)
                dstv = vT_sb[:].rearrange("p (j c) -> p j c", c=64)
                nc.vector.tensor_copy(
                    dstv[:, half * 16:(half + 1) * 16, 0:32], src)

            # ---- prologue: only what super 0/1 needs ----
            for c in range(4):
                emit_proj(wk_sb, bk_sb, k_sb, c, f"pp_k{c}")
            emit_proj(wq_sb, bq_sb, q_sb, 0, "pp_q0")
            emit_vt(0)

            if debug:
                nc.sync.dma_start(dbg["q"][:], q_sb[:])
                nc.sync.dma_start(dbg["k"][:], k_sb[:])
                nc.sync.dma_start(dbg["vT"][:], vT_sb[:])

            # ---- attention, software-pipelined over 64 supers ----
            state = {}
            pv_tiles = {}

            def emit_score_exp(jg):
                tb, j = divmod(jg, NSUP)
                if j == 0:
                    pv_tiles[tb] = pspool.tile(
                        [64, 512], F32, tag="pv", bufs=2, name=f"pv_{tb}")
                st = pspool.tile([128, 2048], F32, tag="st", bufs=1,
                                 name=f"st_{jg}")
                tsl = slice(tb * 512, (tb + 1) * 512)
                for g in range(4):
                    sblk = 4 * j + g
                    nc.tensor.matmul(
                        st[:, g * 512:(g + 1) * 512],
                        k_sb[32 * g:32 * (g + 1), 128 * sblk:128 * (sblk + 1)],
                        q_sb[32 * g:32 * (g + 1), tsl],
                        start=True, stop=True,
                        tile_position=(32 * g, 0))
                p_sb = wpool.tile([128, 2048], BF16, tag="p")
                nc.scalar.activation(p_sb[:], st[:], Exp, scale=SCALE2)
                if debug and jg == 0:
                    nc.sync.dma_start(dbg["p"][:], p_sb[:])
                state[jg] = p_sb

            def emit_pv(jg):
                tb, j = divmod(jg, NSUP)
                p_sb = state.pop(jg)
                pv = pv_tiles[tb]
                for g in range(4):
                    sblk = 4 * j + g
                    nc.tensor.matmul(
                        pv[:], vT_sb[:, 64 * sblk:64 * (sblk + 1)],
                        p_sb[:, g * 512:(g + 1) * 512],
                        start=(j == 0 and g == 0), stop=(j == NSUP - 1 and g == 3),
                        skip_group_check=True)
                if j == NSUP - 1:
                    # t-block epilogue + its output-projection chunk (VectorE
                    # + one PE matmul; all off the critical path)
                    a_sb = wpool.tile([64, 512], F32, tag="acc")
                    nc.vector.tensor_copy(a_sb[:], pv[:])
                    if debug and tb == 0:
                        nc.sync.dma_start(dbg["acc"][:], a_sb[:])
                    tsl = slice(tb * 512, (tb + 1) * 512)
                    dcp = wpool.tile([32, 512], F32, tag="dcp")
                    nc.vector.tensor_copy(dcp[:], a_sb[32:64, :])
                    rc = wpool.tile([32, 512], F32, tag="rc")
                    sc = wpool.tile([32, 512], F32, tag="sc")
                    nc.vector.reciprocal_approx_accurate(rc[:], dcp[:], sc[:])
                    nc.vector.tensor_mul(an_sb[:, tsl], a_sb[0:32, :], rc[:])
                    op = scratch(f"pp_o{tb}")
                    nc.tensor.matmul(op[:], wp_sb[:], an_sb[:, tsl],
                                     start=True, stop=True)
                    o_sb = wpool.tile([128, 512], F32, tag="o")
                    nc.vector.tensor_copy(o_sb[:], op[:])
                    nc.sync.dma_start(out_t[:, tsl], o_sb[:])

            for jg in range(NSUP * NSUP):
                emit_score_exp(jg)
                # stagger remaining input prep into the early supers
                if jg == 0:
                    emit_vt(1)
                elif 1 <= jg <= 3:
                    emit_proj(wk_sb, bk_sb, k_sb, jg + 3, f"pp_k{jg + 3}")
                elif jg == 4:
                    emit_proj(wk_sb, bk_sb, k_sb, 7, "pp_k7")
                    emit_proj(wq_sb, bq_sb, q_sb, 1, "pp_q1")
                elif 5 <= jg <= 10:
                    emit_proj(wq_sb, bq_sb, q_sb, jg - 3, f"pp_q{jg - 3}")
                if jg >= 1:
                    emit_pv(jg - 1)
            emit_pv(NSUP * NSUP - 1)
            if debug:
                nc.sync.dma_start(dbg["an"][:], an_sb[:])

    nc.compile()
    return nc


def _get_nc(debug=False):
    key = ("nc", debug)
    if key not in _cache:
        _cache[key] = _build_nc(debug)
    return _cache[key]


def _make_in_maps(x_, w_qkv, b_qkv, w_proj):
    bf16 = ml_dtypes.bfloat16
    in_maps = []
    for core in range(N_CORES):
        b, g = divmod(core, NH)
        wq = w_qkv[96 * g:96 * g + 32]
        wk = w_qkv[96 * g + 32:96 * g + 64]
        wv = w_qkv[96 * g + 64:96 * g + 96]
        in_maps.append({
            "x": x_[b].astype(bf16),
            "wqT": np.ascontiguousarray(np.tile(wq, (4, 1)).T).astype(bf16),
            "wkT": np.ascontiguousarray(np.tile(wk, (4, 1)).T).astype(bf16),
            "wvT": np.ascontiguousarray(wv.T).astype(bf16),
            "wpT": np.ascontiguousarray(
                w_proj[:, 32 * g:32 * (g + 1)].T).astype(bf16),
            "bq": np.ascontiguousarray(
                np.tile(b_qkv[96 * g:96 * g + 32], 4).reshape(128, 1)),
            "bk": np.ascontiguousarray(
                np.tile(b_qkv[96 * g + 32:96 * g + 64], 4).reshape(128, 1)),
        })
    return in_maps


def _run(x, w_qkv, b_qkv, w_proj, b_proj, trace=False):
    from concourse.bass_utils import run_bass_kernel_spmd

    nc = _get_nc()
    x_ = np.ascontiguousarray(np.asarray(x, np.float32).reshape(B, C, T))
    w_qkv = np.asarray(w_qkv, np.float32)
    b_qkv = np.asarray(b_qkv, np.float32)
    w_proj = np.asarray(w_proj, np.float32)
    b_proj = np.asarray(b_proj, np.float32)

    in_maps = _make_in_maps(x_, w_qkv, b_qkv, w_proj)
    res = run_bass_kernel_spmd(nc, in_maps, core_ids=list(range(N_CORES)),
                               trace=trace)
    out = np.empty((B, C, T), np.float32)
    for b in range(B):
        acc = x_[b] + b_proj[:, None]
        for g in range(NH):
            wp = w_proj[:, 32 * g:32 * (g + 1)]
            bv = b_qkv[96 * g + 64:96 * g + 96]
            acc = acc + res.results[NH * b + g]["out"] + (wp @ bv)[:, None]
        out[b] = acc
    return out.reshape(B, C, Hh, Ww), res


def kernel(x, w_qkv, b_qkv, w_proj, b_proj):
    out, _ = _run(x, w_qkv, b_qkv, w_proj, b_proj, trace=False)
    return out.astype(np.asarray(x).dtype)


# revision 23
# speedup vs baseline: 1.3143x; 1.0471x over previous
"""Trainium2 Bass kernel for nn_Attention2d (sparse_attention).

Math (per reference):
  x: (2, 128, 64, 64); T = 4096 tokens; 4 heads x 32 channels.
  qkv 1x1-conv -> per-head attention over T -> 1x1-conv out proj -> residual.

Sharding: one (batch, head) pair per core (8 cores). Each core computes its
head's attention fully on-chip (flash-style streaming; no max-subtraction --
scores are O(7) so plain exp is safe) and returns the per-head partial of
the output projection; the host sums the 4 head partials per batch and adds
the residual + biases (exact).

Per-core structure (T=4096, t-blocks of 512, s-supers of 4x128):
  - q,k replicated x4 across partition groups so the K=32 score matmuls
    row-pack 4-wide into the PE array (tile_position).
  - Score blocks are written to PSUM as bf16: a full super (4 x 128s x 512t)
    fits in 2 banks, so score tiles triple-buffer in 6 banks and ScalarE's
    exp (the pacer, 1 elem/cycle/lane, 16.7M exps/core) never blocks the
    next super's score matmuls.
  - v is produced directly transposed (x_block stationary) with a 32-wide
    ones block appended: each PV matmul also emits the softmax denominator
    on partitions 32..63.
  - PV accumulates a whole t-block (32 matmuls) into a 1-bank fp32 PSUM
    accumulator (2 rotating banks): no VectorE work on the critical path.
  - Projections / vT / output-projection ride inside the super loop via the
    spare score-tag slots, so prologue and tail stay short.
"""

import numpy as np
import ml_dtypes

B, C, Hh, Ww = 2, 128, 64, 64
T = Hh * Ww          # 4096
NH, CH = 4, 32
SCALE2 = float(1.0 / np.sqrt(CH))
N_CORES = 8
NSUP = T // 512      # 8 supers per t-block, 8 t-blocks

_cache = {}


def _build_nc(debug=False):
    import concourse.tile as tile
    from concourse import bacc, mybir

    BF16 = mybir.dt.bfloat16
    F32 = mybir.dt.float32
    Exp = mybir.ActivationFunctionType.Exp

    nc = bacc.Bacc("TRN2", target_bir_lowering=False, debug=False,
                   num_devices=N_CORES)
    dbg = {}
    if debug:
        dbg["q"] = nc.dram_tensor("dq", [128, T], BF16, kind="ExternalOutput")
        dbg["k"] = nc.dram_tensor("dk", [128, T], BF16, kind="ExternalOutput")
        dbg["vT"] = nc.dram_tensor("dvT", [128, 2048], BF16, kind="ExternalOutput")
        dbg["an"] = nc.dram_tensor("dan", [32, T], BF16, kind="ExternalOutput")
        dbg["acc"] = nc.dram_tensor("dacc", [64, 512], F32, kind="ExternalOutput")
        dbg["p"] = nc.dram_tensor("dp", [128, 2048], BF16, kind="ExternalOutput")

    x_in = nc.dram_tensor("x", [128, T], BF16, kind="ExternalInput")
    wq_in = nc.dram_tensor("wqT", [128, 128], BF16, kind="ExternalInput")
    wk_in = nc.dram_tensor("wkT", [128, 128], BF16, kind="ExternalInput")
    wv_in = nc.dram_tensor("wvT", [128, 32], BF16, kind="ExternalInput")
    wp_in = nc.dram_tensor("wpT", [32, 128], BF16, kind="ExternalInput")
    bq_in = nc.dram_tensor("bq", [128, 1], F32, kind="ExternalInput")
    bk_in = nc.dram_tensor("bk", [128, 1], F32, kind="ExternalInput")
    out_t = nc.dram_tensor("out", [128, T], F32, kind="ExternalOutput")

    with tile.TileContext(nc) as tc:
        with (
            tc.tile_pool(name="const", bufs=1) as cpool,
            tc.tile_pool(name="work", bufs=2) as wpool,
            tc.tile_pool(name="psum", bufs=1, space="PSUM") as pspool,
        ):
            x_sb = cpool.tile([128, T], BF16)
            for c in range(4):
                nc.sync.dma_start(x_sb[:, c * 1024:(c + 1) * 1024],
                                  x_in[:, c * 1024:(c + 1) * 1024])
            wq_sb = cpool.tile([128, 128], BF16)
            nc.sync.dma_start(wq_sb[:], wq_in[:])
            wk_sb = cpool.tile([128, 128], BF16)
            nc.sync.dma_start(wk_sb[:], wk_in[:])
            wv_sb = cpool.tile([128, 32], BF16)
            nc.sync.dma_start(wv_sb[:], wv_in[:])
            wp_sb = cpool.tile([32, 128], BF16)
            nc.sync.dma_start(wp_sb[:], wp_in[:])
            bq_sb = cpool.tile([128, 1], F32)
            nc.sync.dma_start(bq_sb[:], bq_in[:])
            bk_sb = cpool.tile([128, 1], F32)
            nc.sync.dma_start(bk_sb[:], bk_in[:])

            q_sb = cpool.tile([128, T], BF16)
            k_sb = cpool.tile([128, T], BF16)
            vT_sb = cpool.tile([128, 64 * (T // 128)], BF16)  # (128, 2048)
            an_sb = cpool.tile([32, T], BF16)

            nc.gpsimd.memset(vT_sb[:], 1.0)

            # 1-bank fp32 ping-pong tiles for projections / vT / out-proj
            def scratch(nm):
                return pspool.tile([128, 512], F32, tag="pp", bufs=2, name=nm)

            def emit_proj(wsb, bsb, dst, c, nm):
                ps = scratch(nm)
                nc.tensor.matmul(ps[:], wsb[:],
                                 x_sb[:, c * 512:(c + 1) * 512],
                                 start=True, stop=True)
                nc.vector.tensor_scalar_add(
                    dst[:, c * 512:(c + 1) * 512], ps[:], bsb[:])

            def emit_vt(half):
                ps = scratch(f"pp_v{half}")
                for j16 in range(16):
                    j = half * 16 + j16
                    nc.tensor.matmul(
                        ps[:, j16 * 32:(j16 + 1) * 32],
                        x_sb[:, j * 128:(j + 1) * 128],
                        wv_sb[:],
                        start=True, stop=True)
                src = ps[:].rearrange("p (j c) -> p j c", c=32)
                dstv = vT_sb[:].rearrange("p (j c) -> p j c", c=64)
                nc.vector.tensor_copy(
                    dstv[:, half * 16:(half + 1) * 16, 0:32], src)

            # ---- prologue: only what super 0 needs ----
            emit_proj(wk_sb, bk_sb, k_sb, 0, "pp_k0")
            emit_proj(wq_sb, bq_sb, q_sb, 0, "pp_q0")
            # zero "weights" for the PE-warmth filler matmuls
            z_sb = cpool.tile([128, 64], BF16)
            nc.gpsimd.memset(z_sb[:], 0.0)

            if debug:
                nc.sync.dma_start(dbg["q"][:], q_sb[:])
                nc.sync.dma_start(dbg["k"][:], k_sb[:])
                nc.sync.dma_start(dbg["vT"][:], vT_sb[:])

            # ---- attention, software-pipelined over 64 supers ----
            state = {}
            pv_tiles = {}

            def emit_score_exp(jg):
                tb, j = divmod(jg, NSUP)
                if j == 0:
                    pv_tiles[tb] = pspool.tile(
                        [64, 512], F32, tag="pv", bufs=2, name=f"pv_{tb}")
                st = pspool.tile([128, 2048], F32, tag="st", bufs=1,
                                 name=f"st_{jg}")
                tsl = slice(tb * 512, (tb + 1) * 512)
                for g in range(4):
                    sblk = 4 * j + g
                    nc.tensor.matmul(
                        st[:, g * 512:(g + 1) * 512],
                        k_sb[32 * g:32 * (g + 1), 128 * sblk:128 * (sblk + 1)],
                        q_sb[32 * g:32 * (g + 1), tsl],
                        start=True, stop=True,
                        tile_position=(32 * g, 0))
                p_sb = wpool.tile([128, 2048], BF16, tag="p")
                nc.scalar.activation(p_sb[:], st[:], Exp, scale=SCALE2)
                if debug and jg == 0:
                    nc.sync.dma_start(dbg["p"][:], p_sb[:])
                state[jg] = p_sb

            def emit_pv(jg):
                tb, j = divmod(jg, NSUP)
                p_sb = state.pop(jg)
                pv = pv_tiles[tb]
                for g in range(4):
                    sblk = 4 * j + g
                    nc.tensor.matmul(
                        pv[:], vT_sb[:, 64 * sblk:64 * (sblk + 1)],
                        p_sb[:, g * 512:(g + 1) * 512],
                        start=(j == 0 and g == 0), stop=(j == NSUP - 1 and g == 3),
                        skip_group_check=True)
                if j == NSUP - 1:
                    # t-block epilogue + its output-projection chunk (VectorE
                    # + one PE matmul; all off the critical path)
                    a_sb = wpool.tile([64, 512], F32, tag="acc")
                    nc.vector.tensor_copy(a_sb[:], pv[:])
                    if debug and tb == 0:
                        nc.sync.dma_start(dbg["acc"][:], a_sb[:])
                    tsl = slice(tb * 512, (tb + 1) * 512)
                    dcp = wpool.tile([32, 512], F32, tag="dcp")
                    nc.vector.tensor_copy(dcp[:], a_sb[32:64, :])
                    rc = wpool.tile([32, 512], F32, tag="rc")
                    sc = wpool.tile([32, 512], F32, tag="sc")
                    nc.vector.reciprocal_approx_accurate(rc[:], dcp[:], sc[:])
                    nc.vector.tensor_mul(an_sb[:, tsl], a_sb[0:32, :], rc[:])
                    op = scratch(f"pp_o{tb}")
                    nc.tensor.matmul(op[:], wp_sb[:], an_sb[:, tsl],
                                     start=True, stop=True)
                    o_sb = wpool.tile([128, 512], F32, tag="o")
                    nc.vector.tensor_copy(o_sb[:], op[:])
                    nc.sync.dma_start(out_t[:, tsl], o_sb[:])

            for jg in range(NSUP * NSUP):
                if jg >= 8:
                    # keep the PE activity monitor busy while the engine
                    # waits for exp(jg-1): accumulate exact zeros into the
                    # live PV accumulator (start=False, zero weights)
                    tbp = (jg - 1) // NSUP
                    for _ in range(2):
                        nc.tensor.matmul(
                            pv_tiles[tbp][:], z_sb[:],
                            x_sb[0:128, 0:512],
                            start=False, stop=False,
                            skip_group_check=True)
                emit_score_exp(jg)
                # stagger remaining input prep into the supers that have
                # slack, one chunk ahead of the super that needs it
                if jg == 0:
                    emit_vt(0)
                if jg == 1:
                    emit_vt(1)
                if 0 <= jg <= 6:
                    emit_proj(wk_sb, bk_sb, k_sb, jg + 1, f"pp_k{jg + 1}")
                if jg % NSUP == 4 and jg < 56:
                    c = jg // NSUP + 1
                    emit_proj(wq_sb, bq_sb, q_sb, c, f"pp_q{c}")
                if jg >= 1:
                    emit_pv(jg - 1)
            emit_pv(NSUP * NSUP - 1)
            if debug:
                nc.sync.dma_start(dbg["an"][:], an_sb[:])

    nc.compile()
    return nc


def _get_nc(debug=False):
    key = ("nc", debug)
    if key not in _cache:
        _cache[key] = _build_nc(debug)
    return _cache[key]


def _make_in_maps(x_, w_qkv, b_qkv, w_proj):
    bf16 = ml_dtypes.bfloat16
    in_maps = []
    for core in range(N_CORES):
        b, g = divmod(core, NH)
        wq = w_qkv[96 * g:96 * g + 32]
        wk = w_qkv[96 * g + 32:96 * g + 64]
        wv = w_qkv[96 * g + 64:96 * g + 96]
        in_maps.append({
            "x": x_[b].astype(bf16),
            "wqT": np.ascontiguousarray(np.tile(wq, (4, 1)).T).astype(bf16),
            "wkT": np.ascontiguousarray(np.tile(wk, (4, 1)).T).astype(bf16),
            "wvT": np.ascontiguousarray(wv.T).astype(bf16),
            "wpT": np.ascontiguousarray(
                w_proj[:, 32 * g:32 * (g + 1)].T).astype(bf16),
            "bq": np.ascontiguousarray(
                np.tile(b_qkv[96 * g:96 * g + 32], 4).reshape(128, 1)),
            "bk": np.ascontiguousarray(
                np.tile(b_qkv[96 * g + 32:96 * g + 64], 4).reshape(128, 1)),
        })
    return in_maps


def _run(x, w_qkv, b_qkv, w_proj, b_proj, trace=False):
    from concourse.bass_utils import run_bass_kernel_spmd

    nc = _get_nc()
    x_ = np.ascontiguousarray(np.asarray(x, np.float32).reshape(B, C, T))
    w_qkv = np.asarray(w_qkv, np.float32)
    b_qkv = np.asarray(b_qkv, np.float32)
    w_proj = np.asarray(w_proj, np.float32)
    b_proj = np.asarray(b_proj, np.float32)

    in_maps = _make_in_maps(x_, w_qkv, b_qkv, w_proj)
    res = run_bass_kernel_spmd(nc, in_maps, core_ids=list(range(N_CORES)),
                               trace=trace)
    out = np.empty((B, C, T), np.float32)
    for b in range(B):
        acc = x_[b] + b_proj[:, None]
        for g in range(NH):
            wp = w_proj[:, 32 * g:32 * (g + 1)]
            bv = b_qkv[96 * g + 64:96 * g + 96]
            acc = acc + res.results[NH * b + g]["out"] + (wp @ bv)[:, None]
        out[b] = acc
    return out.reshape(B, C, Hh, Ww), res


def kernel(x, w_qkv, b_qkv, w_proj, b_proj):
    out, _ = _run(x, w_qkv, b_qkv, w_proj, b_proj, trace=False)
    return out.astype(np.asarray(x).dtype)
